# revision 34
# baseline (speedup 1.0000x reference)
"""NPMLPHead sampling kernel for Trainium2 (Bass/Tile), SPMD over 8 cores.

Strategy v4 (data-parallel over batch; merged-span gathers):
  - B=16 -> 2 images per core; full (tiny) MLP weights replicated per core.
  - Calibrated DMA cost (from HW traces): aggregate busy ~= 7.4ns/descriptor
    + bytes/25.6GB/s per engine, 16 engines/core. Per-element patch gathers
    (1 descriptor per 4B element) are descriptor-bound; full streams are
    byte-bound. Merged spans sit at the optimum (marginal rule: merge
    neighbours with gap < ~47 elements; slightly larger for L0 to cap the
    per-instruction sequencer cost of ~0.6us per dma_start).
  - L0/L1: gather merged spans: sort patch positions, merge close
    neighbours into one contiguous DRAM run, DMA each run for all
    (image, channel) pairs in ONE instruction ([cp, b*ch, len]).
    Patches are then extracted from the SBUF stash with one [128, b*ch]
    copy per (sorted) patch (DVE mostly; Pool/Act top-up), overlapping the
    stream. The host un-permutes the patch axis afterwards.
  - L2 (128 of 1K, C=1024): stream the whole shard with 4KB descriptors
    (cc-chunked, triple-buffered), compute the MLP on ALL positions in the
    native [C, HW] layout (contraction on partitions), select patches at
    the end with a one-hot PE matmul (q lands on partitions via the
    layer-2 stationary operand).
  - Ring/queue discipline (the v2/v3 lessons): t2 chunks go FIRST on the
    scalar HWDGE ring with its L1-span share paced between chunks (no
    tile-pool stalls, chunks never queue behind spans); sync carries the
    bulk of L0 + half of L1 (HWDGE); gpsimd (SWDGE) only a feed-rate
    top-up of L0. L0/L1 weights ride mid-stream on sync.
  - All matmuls float32r; norm = Square+accum -> sqrt -> recip -> mul.
"""

import sys

sys.path.insert(0, "/opt/trn_rl_repo")

import numpy as np

B = 16
N_CORES = 8
B_LOC = B // N_CORES  # 2
P = 128  # NUM_PATCHES
NCD = 256  # MLP width
LEVELS = [(256, 128), (512, 64), (1024, 32)]  # (C, H) per level
EPS = 1e-7

# Span-merge gap thresholds (elements) per gather level, and SBUF budget
# (bytes per partition) the stash must fit in (auto-shrink threshold if a
# pathological idx distribution would blow the budget).
GAP_THR = {0: 80, 1: 48}
STASH_BUDGET = {0: 48 * 1024, 1: 72 * 1024}
# L0 ring pattern (y=sync HWDGE, g=gpsimd SWDGE top-up).
L0_RING_PAT = "yyg"
# L1 rings: spread over all three so ring drains finish together and
# L0/L1 arrivals stay interleaved in time (extraction FIFOs never
# head-of-line block on one level).
L1_RING_PAT = "cgy"
# Extraction engines: first copies spread wide (s early while Act is
# free), bulk on DVE with Pool top-up.
EXTR_HEAD = "vsvg" * 8
EXTR_PATTERN = "vvvvvg"


def _span_runs(idx_sorted, thr):
    """Merge sorted positions into runs with gaps <= thr. Returns
    (runs=[(lo, ln, off)], cov, off_of: q -> concat offset)."""
    uq = np.unique(np.asarray(idx_sorted))
    bounds = []
    lo = hi = int(uq[0])
    for v in uq[1:]:
        v = int(v)
        if v - hi - 1 <= thr:
            hi = v
        else:
            bounds.append((lo, hi))
            lo = hi = v
    bounds.append((lo, hi))
    runs = []
    o = 0
    for lo_, hi_ in bounds:
        runs.append((lo_, hi_ - lo_ + 1, o))
        o += hi_ - lo_ + 1

    def off_of(q):
        for lo_, ln_, ob in runs:
            if lo_ <= q < lo_ + ln_:
                return ob + (q - lo_)
        raise ValueError(q)

    return runs, o, off_of


def _pick_runs(idx_sorted, thr, n_ch, budget):
    while True:
        runs, cov, off_of = _span_runs(idx_sorted, thr)
        if B_LOC * n_ch * cov * 4 <= budget or thr == 0:
            return runs, cov, off_of
        thr = thr // 2 if thr > 4 else 0


def _build(idx_vals):
    """Build the per-core Bass kernel. idx_vals: 3 int arrays of 128 patch
    ids. L0/L1 are gathered in sorted-index order (host un-permutes)."""
    import concourse.bass as bass
    import concourse.tile as tile
    from concourse import bacc, mybir

    f32 = mybir.dt.float32
    fr = mybir.dt.float32r
    AF = mybir.ActivationFunctionType

    nc = bacc.Bacc(None)

    feats, w1s, b1s, w2s, b2s = [], [], [], [], []
    for l, (C, H) in enumerate(LEVELS):
        feats.append(
            nc.dram_tensor(f"feat{l}", [B_LOC, C, H, H], fr, kind="ExternalInput")
        )
        w1s.append(nc.dram_tensor(f"w1_{l}", [C, NCD], fr, kind="ExternalInput"))
        b1s.append(nc.dram_tensor(f"b1_{l}", [NCD], fr, kind="ExternalInput"))
        w2s.append(nc.dram_tensor(f"w2_{l}", [NCD, NCD], fr, kind="ExternalInput"))
        b2s.append(nc.dram_tensor(f"b2_{l}", [NCD], fr, kind="ExternalInput"))
    C2, H2 = LEVELS[2]
    HW2 = H2 * H2  # 1024
    NCH2 = C2 // 128  # 8
    QC2 = HW2 // 128  # 8 q-chunks
    # one-hot select for level 2, pre-transposed on host: [ql, (qc p)]
    oh = nc.dram_tensor("oh2", [128, QC2 * P], fr, kind="ExternalInput")
    out = nc.dram_tensor("out", [3, B_LOC, P, NCD], f32, kind="ExternalOutput")

    sv = {l: np.sort(np.asarray(idx_vals[l]).astype(np.int64)) for l in (0, 1)}
    runs, covs, offf = {}, {}, {}
    for l in (0, 1):
        n_ch = LEVELS[l][0] // 128
        runs[l], covs[l], offf[l] = _pick_runs(
            sv[l], GAP_THR[l], n_ch, STASH_BUDGET[l]
        )

    with tile.TileContext(nc) as tc:
        with (
            tc.tile_pool(name="consts", bufs=1) as consts,
            tc.tile_pool(name="stash", bufs=1) as stash,
            tc.tile_pool(name="t2p", bufs=2) as t2p,
            tc.tile_pool(name="xt", bufs=1) as xtp,
            tc.tile_pool(name="work", bufs=3) as work,
            tc.tile_pool(name="psum", bufs=1, space=bass.MemorySpace.PSUM) as psum,
            tc.tile_pool(name="psum1", bufs=1, space=bass.MemorySpace.PSUM) as psum1,
        ):
            # t2 image-0 half-image DMAs lead the scalar ring: byte-heavy
            # descriptors fill the DMA engines from t~1us
            src2 = [
                feats[2][b].rearrange("(ch cp) h w -> cp ch (h w)", cp=128)
                for b in range(B_LOC)
            ]

            def emit_t2(b, hf):
                t = t2p.tile([128, 4 * HW2], fr, tag="t2c", name=f"t2c{b}_{hf}")
                nc.scalar.dma_start(
                    t[:].rearrange("cp (ch hw) -> cp ch hw", hw=HW2),
                    src2[b][:, hf * 4 : hf * 4 + 4, :],
                )
                return t

            t2ts = {(0, hf): emit_t2(0, hf) for hf in range(2)}

            ones_f = consts.tile([1, 512], f32, tag="ones_f")
            nc.vector.memset(ones_f[:], 1.0)
            ones = consts.tile([1, 512], fr, tag="ones")
            nc.scalar.copy(ones[:], ones_f[:])

            w1_sb, w2_sb, b1_sb, b2_sb = {}, {}, {}, {}

            def load_weights(l, eng):
                C, H = LEVELS[l]
                n_ch = C // 128
                t = consts.tile([128, n_ch * NCD], fr, tag=f"w1_{l}", name=f"w1sb{l}")
                eng.dma_start(
                    t[:].rearrange("cp (ch n) -> cp ch n", n=NCD),
                    w1s[l][:].rearrange("(ch cp) n -> cp ch n", cp=128),
                )
                w1_sb[l] = t
                t = consts.tile([128, 2 * NCD], fr, tag=f"w2_{l}", name=f"w2sb{l}")
                eng.dma_start(
                    t[:].rearrange("cp (ch n) -> cp ch n", n=NCD),
                    w2s[l][:].rearrange("(ch cp) n -> cp ch n", cp=128),
                )
                w2_sb[l] = t
                t = consts.tile([1, NCD], fr, tag=f"b1_{l}", name=f"b1sb{l}")
                eng.dma_start(t[:], b1s[l][:].rearrange("(o n) -> o n", o=1))
                b1_sb[l] = t
                t = consts.tile([1, NCD], fr, tag=f"b2_{l}", name=f"b2sb{l}")
                eng.dma_start(t[:], b2s[l][:].rearrange("(o n) -> o n", o=1))
                b2_sb[l] = t

            load_weights(2, nc.scalar)
            oh_sb = consts.tile([128, QC2 * P], fr, tag="oh2")
            nc.scalar.dma_start(oh_sb[:], oh[:])

            # --- L2 layer 1 (full-compute on all positions) ---
            def emit_l2_layer1(b):
                gs = [
                    psum1.tile([128, 512], f32, tag=f"g{j}", name=f"g{j}_{b}")
                    for j in range(4)
                ]
                for hf in range(2):
                    tc2t = t2ts[(b, hf)]
                    for cl in range(4):
                        cc = hf * 4 + cl
                        for half in range(2):
                            for qn in range(2):
                                nc.tensor.matmul(
                                    gs[half * 2 + qn][:],
                                    w1_sb[2][
                                        :,
                                        cc * NCD
                                        + half * 128 : cc * NCD
                                        + half * 128
                                        + 128,
                                    ],
                                    tc2t[
                                        :,
                                        cl * HW2
                                        + qn * 512 : cl * HW2
                                        + qn * 512
                                        + 512,
                                    ],
                                    start=(cc == 0),
                                    stop=False,
                                )
                h2 = xtp.tile([128, 2 * HW2], fr, tag=f"h2_{b}", name=f"h2_{b}")
                for half in range(2):
                    for qn in range(2):
                        nc.tensor.matmul(  # + b1 broadcast over all q
                            gs[half * 2 + qn][:],
                            b1_sb[2][0:1, half * 128 : half * 128 + 128],
                            ones[0:1, 0:512],
                            start=False,
                            stop=True,
                        )
                        nc.scalar.activation(
                            h2[
                                :,
                                (half * 2 + qn) * 512 : (half * 2 + qn) * 512
                                + 512,
                            ],
                            gs[half * 2 + qn][:],
                            AF.Relu,
                        )
                return h2

            # image 0 compute first (its matmuls/relus release the t2
            # pool slots), then image 1's t2 DMAs near the ring head so
            # feat2 is fully on chip early — the L2 dependency train
            # (layer1 -> relu -> ksb -> select) is the longest chain.
            h2s = [emit_l2_layer1(0)]
            for hf in range(2):
                t2ts[(1, hf)] = emit_t2(1, hf)

            # --- L0/L1 merged-span gather setup ---
            src_sp, stv, xts = {}, {}, {}
            for l in (0, 1):
                C, H = LEVELS[l]
                n_ch = C // 128
                src_sp[l] = feats[l][:].rearrange(
                    "b (ch cp) h w -> cp (b ch) (h w)", cp=128
                )
                t = stash.tile(
                    [128, B_LOC * n_ch * covs[l]], fr, tag=f"st{l}", name=f"st{l}"
                )
                stv[l] = t[:].rearrange("c (bc v) -> c bc v", v=covs[l])
                xts[l] = xtp.tile(
                    [128, B_LOC * n_ch * P], fr, tag=f"xt_{l}", name=f"xt{l}"
                )

            rings = {"y": nc.sync, "c": nc.scalar, "g": nc.gpsimd}
            # merged emission list (L0/L1 interleaved by fractional position)
            em = []
            n0, n1 = len(runs[0]), len(runs[1])
            for i, r in enumerate(runs[0]):
                em.append((i / n0, 0, r, L0_RING_PAT[i % len(L0_RING_PAT)]))
            for j, r in enumerate(runs[1]):
                em.append((j / n1, 1, r, L1_RING_PAT[j % len(L1_RING_PAT)]))
            em.sort(key=lambda e: e[0])

            def emit_span(e):
                _, l, (lo, ln, off), key = e
                rings[key].dma_start(
                    stv[l][:, :, off : off + ln],
                    src_sp[l][:, :, lo : lo + ln],
                )

            # all spans in merged order (each ring FIFO keeps relative
            # order; scalar's land behind the 4 t2 DMAs). L0/L1 weights
            # ride the sync ring mid-stream.
            with nc.allow_non_contiguous_dma("merged-span patch gather"):
                for e in em[:12]:
                    emit_span(e)
                load_weights(0, nc.sync)
                load_weights(1, nc.sync)
                for e in em[12:]:
                    emit_span(e)

            # --- L2 layer 1 for image 1 (its t2 DMAs are near the ring
            # head, so this runs ~t=30-40, not in the tail) ---
            h2s.append(emit_l2_layer1(1))

            # --- L2 layer 2 + one-hot select (early: only needs h2/w2/oh).
            # ksb copies ride scalar (PSUM-capable; its queue is clear by
            # then); PE emission is software-pipelined (k(qc+1) ahead of
            # py(qc)) so the PE never idles on a ksb in flight. The norm
            # (needs DVE reciprocal) is deferred past extraction so the
            # vector FIFO stays clear for extraction copies. ---
            ysums = []
            for b in range(B_LOC):
                h2 = h2s[b]
                py = psum.tile([128, NCD], f32, tag=f"pyb{b}", name=f"py2_{b}")
                ksbs = {}

                def emit_k(qc, b=b, h2=h2, ksbs=ksbs):
                    k = psum.tile([128, NCD], f32, tag=f"k{qc % 2}", name="k")
                    for half in range(2):
                        o = (half * 2 + qc // 4) * 512 + (qc % 4) * 128
                        nc.tensor.matmul(
                            k[:],
                            h2[:, o : o + 128],
                            w2_sb[2][:, half * NCD : (half + 1) * NCD],
                            start=(half == 0),
                            stop=False,
                        )
                    nc.tensor.matmul(  # + b2 for every q (select sums to 1)
                        k[:],
                        ones[0:1, 0:128],
                        b2_sb[2][0:1, :],
                        start=False,
                        stop=True,
                    )
                    ksb = work.tile([128, NCD], fr, tag="ksb", name="ksb")
                    nc.scalar.copy(ksb[:], k[:])
                    ksbs[qc] = ksb

                def emit_py(qc, b=b, py=py, ksbs=ksbs):
                    nc.tensor.matmul(
                        py[:],
                        oh_sb[:, qc * P : (qc + 1) * P],
                        ksbs[qc][:],
                        start=(qc == 0),
                        stop=(qc == QC2 - 1),
                    )

                emit_k(0)
                for qc in range(1, QC2):
                    emit_k(qc)
                    emit_py(qc - 1)
                emit_py(QC2 - 1)
                ysums.append(py)

            # --- extraction: one [128, b*ch] copy per sorted patch,
            # overlapping the span stream; L0/L1 interleaved ---
            xtv = {
                l: xts[l][:].rearrange("c (b ch p) -> c b ch p", b=B_LOC, p=P)
                for l in (0, 1)
            }
            stv4 = {
                l: stv[l].rearrange("c (b ch) v -> c b ch v", b=B_LOC)
                for l in (0, 1)
            }
            engs = {"v": nc.vector, "s": nc.scalar, "g": nc.gpsimd}
            ei = 0
            for i in range(P):
                for l in (0, 1):
                    o = offf[l](int(sv[l][i]))
                    if ei < len(EXTR_HEAD):
                        key = EXTR_HEAD[ei]
                    else:
                        key = EXTR_PATTERN[ei % len(EXTR_PATTERN)]
                    eng = engs[key]
                    ei += 1
                    if eng is nc.scalar:
                        eng.copy(xtv[l][:, :, :, i], stv4[l][:, :, :, o])
                    else:
                        eng.tensor_copy(xtv[l][:, :, :, i], stv4[l][:, :, :, o])

            for b in range(B_LOC):
                _norm_and_store(nc, work, AF, f32, ysums[b], out, 2, b)

            # --- MLP for L0/L1 (both images batched into N=256) ---
            for l in (0, 1):
                C, H = LEVELS[l]
                n_ch = C // 128
                x4 = xts[l][:].rearrange("c (b ch p) -> c ch b p", b=B_LOC, p=P)
                hts = []
                for half in range(2):
                    ph = psum1.tile([128, 512], f32, tag=f"g{half}", name="ph")
                    for ch in range(n_ch):
                        o = ch * NCD + half * 128
                        nc.tensor.matmul(
                            ph[:, 0 : B_LOC * P],
                            w1_sb[l][:, o : o + 128],
                            x4[:, ch],
                            start=(ch == 0),
                            stop=False,
                        )
                    nc.tensor.matmul(  # + b1 (rank-1)
                        ph[:, 0 : B_LOC * P],
                        b1_sb[l][0:1, half * 128 : half * 128 + 128],
                        ones[0:1, 0 : B_LOC * P],
                        start=False,
                        stop=True,
                    )
                    ht = work.tile([128, B_LOC * P], fr, tag="ht", name="ht")
                    nc.scalar.activation(ht[:], ph[:, 0 : B_LOC * P], AF.Relu)
                    hts.append(ht)

                for b in range(B_LOC):
                    # reuse the k0/k1 banks (free after L2 layer 2) so the
                    # MLP never waits on the deferred L2 norm reads
                    py = psum.tile([128, NCD], f32, tag=f"k{b}", name="py")
                    for half in range(2):
                        nc.tensor.matmul(
                            py[:],
                            hts[half][:, b * P : (b + 1) * P],
                            w2_sb[l][:, half * NCD : (half + 1) * NCD],
                            start=(half == 0),
                            stop=False,
                        )
                    nc.tensor.matmul(  # + b2 (rank-1)
                        py[:],
                        ones[0:1, 0:P],
                        b2_sb[l][0:1, :],
                        start=False,
                        stop=True,
                    )
                    _norm_and_store(nc, work, AF, f32, py, out, l, b)

    nc.compile()
    return nc


def _norm_and_store(nc, work, AF, f32, py, out, l, b):
    # l2-normalize rows: Square+accum -> sqrt -> reciprocal(DVE) -> mul
    sq = work.tile([128, NCD], f32, tag="sq", name="sq")
    ssq = work.tile([128, 1], f32, tag="ssq", name="ssq")
    nc.scalar.activation(sq[:], py[:], AF.Square, accum_out=ssq[:])
    inv = work.tile([128, 1], f32, tag="inv", name="inv")
    nrm = work.tile([128, 1], f32, tag="nrm", name="nrm")
    nc.scalar.sqrt(nrm[:], ssq[:])
    nc.vector.reciprocal(inv[:], nrm[:])
    yo = work.tile([128, NCD], f32, tag="yo", name="yo")
    nc.scalar.mul(yo[:], py[:], inv[:])
    store_eng = [nc.sync, nc.gpsimd][(2 * l + b) % 2]
    store_eng.dma_start(out[l, b], yo[:])


def _run(inputs, trace=False):
    from concourse.bass_utils import run_bass_kernel_spmd

    feats = [
        np.ascontiguousarray(np.asarray(inputs[f"feat{l}"], dtype=np.float32))
        for l in range(3)
    ]
    idxs = [np.asarray(inputs[f"idx{l}"]).astype(np.int64) for l in range(3)]
    nc = _build(idxs)

    oh2 = np.zeros((128, 8 * P), np.float32)
    for p, q in enumerate(idxs[2]):
        oh2[int(q) % 128, (int(q) // 128) * P + p] = 1.0

    in_maps = []
    for c in range(N_CORES):
        m = {"oh2": oh2}
        for l in range(3):
            m[f"feat{l}"] = feats[l][c * B_LOC : (c + 1) * B_LOC]
            m[f"w1_{l}"] = np.asarray(inputs[f"w1_{l}"], dtype=np.float32)
            m[f"b1_{l}"] = np.asarray(inputs[f"b1_{l}"], dtype=np.float32)
            m[f"w2_{l}"] = np.asarray(inputs[f"w2_{l}"], dtype=np.float32)
            m[f"b2_{l}"] = np.asarray(inputs[f"b2_{l}"], dtype=np.float32)
        in_maps.append(m)

    res = run_bass_kernel_spmd(
        nc, in_maps, core_ids=list(range(N_CORES)), trace=trace
    )
    full = np.concatenate([r["out"] for r in res.results], axis=1)
    # levels 0/1 were gathered in sorted-index order; un-permute patches
    for l in (0, 1):
        order = np.argsort(idxs[l], kind="stable")
        unperm = np.empty_like(full[l])
        unperm[:, order, :] = full[l]
        full[l] = unperm
    return full.astype(np.float32), res


def kernel(**inputs) -> np.ndarray:
    out, _ = _run(inputs, trace=False)
    return out
